# revision 29
# baseline (speedup 1.0000x reference)
"""Trainium2 Bass kernel for nn_GCFNN (2-modality GCN+GAT VAE-ish net).

Strategy: row-shard the node dim N=4096 across 8 cores (512 rows each).
Each core holds adj[rows_c].T ([4096, 512], node-j on partitions) resident in
SBUF in fp16. Per GCN layer: cores compute their support slice x_c @ W (fp16),
AllGather it ([N, 256] fp16), then aggregate transposed:
    x1T[f, i] = sum_j s[j, f] * adjT[j, i]
so the bias add + leaky-relu fuse into one ScalarE Prelu. GAT attention is
computed with logits transposed (eT[j, i]); the per-j a2h projection rides in
the h_aug gather's spare column (no separate collective), masked-exp runs as
DVE add + DVE lrelu(max) + Act exp(l-4) [softmax-shift-invariant, fp16-safe]
+ Pool mask-mult, and the masked-exp output pm is directly the lhsT of att@h.
The joint product-of-experts batches both modalities on partitions; the three
predictor MLPs share one fused softmax via indicator-matrix matmuls. The
[15, 512] per-core result is transposed host-side.
"""

import functools
import os
import sys

import numpy as np

if "/opt/trn_rl_repo" not in sys.path:
    sys.path.insert(0, "/opt/trn_rl_repo")

import concourse.bacc as bacc
import concourse.mybir as mybir
import concourse.tile as tile
from concourse.bass_interp import get_hw_module
from concourse.bass_utils import run_bass_kernel_spmd

N, D, H, F2, Z, Y, PH, M = 4096, 400, 256, 128, 64, 5, 128, 2
NCORES = 8
S = N // NCORES          # 512 rows per core
NJT = N // 128           # 32 j-tiles
NIB = S // 128           # 4 i-blocks
DP = 512                 # D padded to 4*128
NDT = DP // 128          # 4 d-tiles
NFB = H // 128           # 2 feature blocks
HA = 130                 # h_aug cols: 128 h + 1 ones + 1 a2h
ESHIFT = -4.0            # exp(l + ESHIFT): softmax-invariant, keeps fp16 small
EPS = 1e-8

F32 = mybir.dt.float32
F32R = mybir.dt.float32r
F16 = mybir.dt.float16

AFT = mybir.ActivationFunctionType
ALU = mybir.AluOpType


def _emit(nc, tc, P):
    """Emit the whole per-core program. P = dict of DRAM param APs."""
    ag1_in, ag1_out, ag2_in, ag2_out = [], [], [], []
    agh_in, agh_out = [], []
    for m in range(M):
        ag1_in.append(nc.dram_tensor(f"ag1_in{m}", [S, H], F16))
        ag1_out.append(nc.dram_tensor(f"ag1_out{m}", [N, H], F16, addr_space="Shared"))
        ag2_in.append(nc.dram_tensor(f"ag2_in{m}", [S, H], F16))
        ag2_out.append(nc.dram_tensor(f"ag2_out{m}", [N, H], F16, addr_space="Shared"))
        agh_in.append(nc.dram_tensor(f"agh_in{m}", [S, HA], F16))
        agh_out.append(nc.dram_tensor(f"agh_out{m}", [N, HA], F16, addr_space="Shared"))

    rg = [list(range(NCORES))]

    with (
        tc.tile_pool(name="persist", bufs=1) as pp,
        tc.tile_pool(name="stream", bufs=5) as sp,
        tc.tile_pool(name="work", bufs=3) as wp,
        tc.tile_pool(name="att16", bufs=4) as ap16,
        tc.tile_pool(name="pmpool", bufs=3) as pmp,
        tc.tile_pool(name="small", bufs=4) as smp,
        tc.tile_pool(name="ps512", bufs=4, space="PSUM") as ps512,
        tc.tile_pool(name="ps256", bufs=2, space="PSUM") as ps256,
        tc.tile_pool(name="pssm", bufs=2, space="PSUM") as pssm,
    ):
        # ---------- persistent loads ----------
        adjT, W1, W2, Wg, b1, b2, ga = [], [], [], [], [], [], []
        for m in range(M):
            t = pp.tile([128, NJT * S], F16, tag=f"adjT{m}", name=f"adjT{m}")
            for j in range(NJT):
                nc.sync.dma_start(
                    out=t[:, j * S:(j + 1) * S],
                    in_=P[f"adjT{m}"][j * 128:(j + 1) * 128, :],
                )
            adjT.append(t)

            t = pp.tile([128, NDT * H], F16, tag=f"W1_{m}", name=f"W1_{m}")
            for k in range(NDT):
                nc.sync.dma_start(
                    out=t[:, k * H:(k + 1) * H],
                    in_=P[f"gc1_W{m}"][k * 128:(k + 1) * 128, :],
                )
            W1.append(t)

            t = pp.tile([128, NFB * H], F16, tag=f"W2_{m}", name=f"W2_{m}")
            for k in range(NFB):
                nc.sync.dma_start(
                    out=t[:, k * H:(k + 1) * H],
                    in_=P[f"gc2_W{m}"][k * 128:(k + 1) * 128, :],
                )
            W2.append(t)

            t = pp.tile([128, NFB * F2], F32R, tag=f"Wg_{m}", name=f"Wg_{m}")
            nc.sync.dma_start(
                out=t[:].rearrange("p (t f) -> p t f", t=NFB),
                in_=P[f"gat_W{m}"].rearrange("(t p) f -> p t f", p=128),
            )
            Wg.append(t)

            t = pp.tile([128, NFB], F32, tag=f"b1_{m}", name=f"b1_{m}")
            nc.sync.dma_start(
                out=t[:].rearrange("p (t o) -> p t o", t=NFB),
                in_=P[f"gc1_b{m}"].rearrange("(t p) o -> p t o", p=128),
            )
            b1.append(t)

            t = pp.tile([128, NFB], F32, tag=f"b2_{m}", name=f"b2_{m}")
            nc.sync.dma_start(
                out=t[:].rearrange("p (t o) -> p t o", t=NFB),
                in_=P[f"gc2_b{m}"].rearrange("(t p) o -> p t o", p=128),
            )
            b2.append(t)

            t = pp.tile([128, 2], F32, tag=f"ga_{m}", name=f"ga_{m}")
            nc.sync.dma_start(
                out=t[:].rearrange("p (t o) -> p t o", t=2),
                in_=P[f"gat_a{m}"].rearrange("(t p) o -> p t o", p=128),
            )
            t16 = pp.tile([128, 2], F16, tag=f"ga16_{m}", name=f"ga16_{m}")
            nc.vector.tensor_copy(t16[:], t[:])
            ga.append((t, t16))

        # predictor weights: order j(=joint), 0, 1 -> output rows 0:5, 5:10, 10:15
        spW1, spb1, spW2, spb2 = [], [], [], []
        for k, key in enumerate((("jpW1", "jpb1", "jpW2", "jpb2"),
                                 ("spW1_0", "spb1_0", "spW2_0", "spb2_0"),
                                 ("spW1_1", "spb1_1", "spW2_1", "spb2_1"))):
            t = pp.tile([Z, PH], F16, tag=f"spW1{k}", name=f"spW1{k}")
            nc.sync.dma_start(out=t[:], in_=P[key[0]][:, :])
            spW1.append(t)
            t = pp.tile([PH, 1], F32, tag=f"spb1{k}", name=f"spb1{k}")
            nc.sync.dma_start(out=t[:], in_=P[key[1]][:, :])
            spb1.append(t)
            t = pp.tile([PH, Y], F16, tag=f"spW2{k}", name=f"spW2{k}")
            nc.sync.dma_start(out=t[:], in_=P[key[2]][:, :])
            spW2.append(t)
            t = pp.tile([Y, 1], F32, tag=f"spb2{k}", name=f"spb2{k}")
            nc.sync.dma_start(out=t[:], in_=P[key[3]][:, :])
            spb2.append(t)

        ones_row = pp.tile([1, 128], F16, tag="ones_row")
        nc.vector.memset(ones_row[:], 1.0)
        eshift_b = pp.tile([128, 1], F32, tag="eshift_b")
        nc.vector.memset(eshift_b[:], ESHIFT)
        # predictor-softmax indicator mats (host constants): predictors live
        # at partition bases 0/32/64 of a [69, S] stack
        selDown = pp.tile([69, 3], F16, tag="selDown")
        nc.sync.dma_start(out=selDown[:], in_=P["selDown"][:, :])
        selUp = pp.tile([3, 69], F16, tag="selUp")
        nc.sync.dma_start(out=selUp[:], in_=P["selUp"][:, :])

        # per-modality persistent intermediates
        x1T = [pp.tile([128, NFB * S], F16, tag=f"x1T{m}", name=f"x1T{m}") for m in range(M)]
        x2T = [pp.tile([128, NFB * S], F32R, tag=f"x2T{m}", name=f"x2T{m}") for m in range(M)]
        hT = [pp.tile([128, S], F16, tag=f"hT{m}", name=f"hT{m}") for m in range(M)]
        a1hb = [pp.tile([128, S], F16, tag=f"a1hb{m}", name=f"a1hb{m}") for m in range(M)]
        a2hb = [pp.tile([128, NJT], F32, tag=f"a2hb{m}", name=f"a2hb{m}") for m in range(M)]
        # attention out, batched across modalities on partitions:
        # attMU rows 64m:64m+64 = mu_m ; attLV rows 64m:64m+64 = logvar_m
        attMU = pp.tile([128, S], F16, tag="attMU")
        attLV = pp.tile([128, S], F16, tag="attLV")

        # ---------- stage A: support1 = x @ W1 (sharded), AllGather ----------
        def stage_A(m):
            for ib in range(NIB):
                ps = ps256.tile([128, H], F32, tag="psA")
                for k in range(NDT):
                    xt = sp.tile([128, 128], F16, tag="xst", bufs=5)
                    nc.sync.dma_start(
                        out=xt[:],
                        in_=P[f"xT{m}"][k * 128:(k + 1) * 128,
                                        ib * 128:(ib + 1) * 128],
                    )
                    nc.tensor.matmul(
                        ps[:], xt[:], W1[m][:, k * H:(k + 1) * H],
                        start=(k == 0), stop=(k == NDT - 1),
                    )
                sb = wp.tile([128, H], F16, tag="scp")
                nc.vector.tensor_copy(sb[:], ps[:])
                nc.sync.dma_start(out=ag1_in[m][ib * 128:(ib + 1) * 128, :], in_=sb[:])
            nc.gpsimd.collective_compute(
                "AllGather", ALU.bypass, replica_groups=rg,
                ins=[ag1_in[m].ap().opt()], outs=[ag1_out[m].ap().opt()],
            )

        # ---------- aggregation: outT = prelu(adj @ s + b), transposed -------
        def stage_agg(m, ag_out, bias, outT):
            psf = [ps512.tile([128, S], F32, tag="psAgg", name=f"psf{i}")
                   for i in range(NFB)]
            for j in range(NJT):
                st = sp.tile([128, H], F16, tag="sstream", bufs=8)
                nc.sync.dma_start(out=st[:], in_=ag_out[j * 128:(j + 1) * 128, :])
                for fb in range(NFB):
                    nc.tensor.matmul(
                        psf[fb][:],
                        st[:, fb * 128:(fb + 1) * 128],
                        adjT[m][:, j * S:(j + 1) * S],
                        start=(j == 0), stop=(j == NJT - 1),
                    )
            for fb in range(NFB):
                nc.scalar.activation(
                    outT[:, fb * S:(fb + 1) * S], psf[fb][:],
                    AFT.Prelu, bias=bias[:, fb:fb + 1], scale=1.0, alpha=0.25,
                )

        # ---------- stage C: support2 = x1 @ W2 (sharded), AllGather ---------
        def stage_C(m):
            for ib in range(NIB):
                ps = ps256.tile([128, H], F32, tag="psA")
                for fb in range(NFB):
                    nc.tensor.matmul(
                        ps[:],
                        x1T[m][:, fb * S + ib * 128: fb * S + (ib + 1) * 128],
                        W2[m][:, fb * H:(fb + 1) * H],
                        start=(fb == 0), stop=(fb == NFB - 1),
                    )
                sb = wp.tile([128, H], F16, tag="scp")
                nc.vector.tensor_copy(sb[:], ps[:])
                nc.sync.dma_start(out=ag2_in[m][ib * 128:(ib + 1) * 128, :], in_=sb[:])
            nc.gpsimd.collective_compute(
                "AllGather", ALU.bypass, replica_groups=rg,
                ins=[ag2_in[m].ap().opt()], outs=[ag2_out[m].ap().opt()],
            )

        # ---------- stage E: h, hT, a-projections, AllGather h_aug ----------
        def stage_E(m):
            # hT[g, i] = sum_f Wg[f, g] x2T[f, i]
            pshT = ps512.tile([128, S], F32, tag="psAgg")
            for fb in range(NFB):
                nc.tensor.matmul(
                    pshT[:],
                    Wg[m][:, fb * F2:(fb + 1) * F2],
                    x2T[m][:, fb * S:(fb + 1) * S],
                    start=(fb == 0), stop=(fb == NFB - 1),
                )
            nc.vector.tensor_copy(hT[m][:], pshT[:])

            # a1h broadcast [128, S]: lhsT = a1 replicated over free dim
            a1rep = smp.tile([128, 128], F16, tag="smh")
            nc.vector.memset(a1rep[:], 0.0)
            nc.vector.tensor_scalar_add(a1rep[:], a1rep[:], ga[m][0][:, 0:1])
            psa1 = ps512.tile([128, S], F32, tag="psAgg")
            nc.tensor.matmul(psa1[:], a1rep[:], hT[m][:], start=True, stop=True)
            nc.vector.tensor_copy(a1hb[m][:], psa1[:])

            # a2h per i-block + h blocks -> h_aug bounce (a2h rides col 129)
            psa2 = pssm.tile([128, NIB], F32, tag="sm")
            hcat = wp.tile([128, NIB * HA], F16, tag="hcat", bufs=1)
            for ib in range(NIB):
                nc.tensor.matmul(
                    psa2[:, ib:ib + 1],
                    hT[m][:, ib * 128:(ib + 1) * 128],
                    ga[m][1][:, 1:2],
                    start=True, stop=True,
                )
                psh = ps256.tile([128, F2], F32, tag="psA")
                for fb in range(NFB):
                    nc.tensor.matmul(
                        psh[:],
                        x2T[m][:, fb * S + ib * 128: fb * S + (ib + 1) * 128],
                        Wg[m][:, fb * F2:(fb + 1) * F2],
                        start=(fb == 0), stop=(fb == NFB - 1),
                    )
                nc.vector.tensor_copy(hcat[:, ib * HA: ib * HA + F2], psh[:])
                nc.vector.memset(hcat[:, ib * HA + F2: ib * HA + F2 + 1], 1.0)
                nc.vector.tensor_copy(
                    hcat[:, ib * HA + F2 + 1: ib * HA + HA], psa2[:, ib:ib + 1],
                )
            for ib in range(NIB):
                nc.sync.dma_start(
                    out=agh_in[m][ib * 128:(ib + 1) * 128, :],
                    in_=hcat[:, ib * HA:(ib + 1) * HA],
                )
            nc.gpsimd.collective_compute(
                "AllGather", ALU.bypass, replica_groups=rg,
                ins=[agh_in[m].ap().opt()], outs=[agh_out[m].ap().opt()],
            )

        # ---------- stage F: masked attention, transposed ----------
        def stage_F(m):
            psO = ps512.tile([128, S], F32, tag="psAgg")
            psden = pssm.tile([1, S], F32, tag="sm")
            for j in range(NJT):
                ht = sp.tile([128, HA], F16, tag="haugst", bufs=6)
                nc.sync.dma_start(out=ht[:], in_=agh_out[m][j * 128:(j + 1) * 128, :])
                # a2h bias column to f32 (DVE scalar operands must be f32)
                nc.vector.tensor_copy(a2hb[m][:, j:j + 1], ht[:, HA - 1:HA])
                # v = a1h_i + a2h_j ; l = lrelu(v) = max(v, .25v)  (DVE fp16)
                v = ap16.tile([128, S], F16, tag="attv")
                nc.vector.tensor_scalar_add(v[:], a1hb[m][:], a2hb[m][:, j:j + 1])
                l = ap16.tile([128, S], F16, tag="attl")
                nc.vector.scalar_tensor_tensor(
                    out=l[:], in0=v[:], scalar=0.25, in1=v[:],
                    op0=ALU.mult, op1=ALU.max,
                )
                # p = exp(l - 4)  (Act; softmax-shift-invariant)
                p = ap16.tile([128, S], F16, tag="attp")
                nc.scalar.activation(p[:], l[:], AFT.Exp, bias=eshift_b[:, 0:1],
                                     scale=1.0)
                # pm = (adj > 0) * p  (Pool)
                pm = pmp.tile([128, S], F16, tag="att_pm", bufs=3)
                nc.vector.scalar_tensor_tensor(
                    out=pm[:], in0=adjT[m][:, j * S:(j + 1) * S],
                    scalar=0.0, in1=p[:], op0=ALU.is_gt, op1=ALU.mult,
                )
                nc.tensor.matmul(
                    psO[:], ht[:, 0:F2], pm[:],
                    start=(j == 0), stop=(j == NJT - 1),
                )
                nc.tensor.matmul(
                    psden[:], ht[:, F2:F2 + 1], pm[:],
                    start=(j == 0), stop=(j == NJT - 1),
                )
            rec = smp.tile([1, S], F16, tag="smr")
            with nc.allow_low_precision(reason="softmax denom fp16 ~5e-4 rel"):
                nc.vector.reciprocal(rec[:], psden[:])
            psR = ps512.tile([128, S], F32, tag="psAgg")
            nc.tensor.matmul(psR[:], ones_row[:, :], rec[:], start=True, stop=True)
            Rsb = wp.tile([128, S], F32, tag="w512")
            nc.vector.tensor_copy(Rsb[:], psR[:])
            sc = wp.tile([128, S], F32, tag="w512")
            nc.vector.tensor_tensor(out=sc[:], in0=psO[:], in1=Rsb[:], op=ALU.mult)
            nc.scalar.activation(attMU[Z * m:Z * (m + 1), :], sc[0:Z, :],
                                 AFT.Prelu, alpha=0.25)
            nc.scalar.activation(attLV[Z * m:Z * (m + 1), :], sc[Z:2 * Z, :],
                                 AFT.Prelu, alpha=0.25)

        # ---------- stage G: joint PoE (batched) + fused predictors ----------
        def stage_G():
            # masks for both modalities, batched on partitions [128, S]
            MsbB = smp.tile([128, S], F32, tag="sm", name="MsbB")
            for m in range(M):
                nc.sync.dma_start(
                    out=MsbB[Z * m:Z * (m + 1), :],
                    in_=P["maskT"][0:1, m * S:(m + 1) * S].to_broadcast((Z, S)),
                )
            # pm_m = mask_m / (exp(lv_m) + EPS), both modalities at once
            eB = smp.tile([128, S], F32, tag="sm", name="poe_e")
            nc.scalar.activation(eB[:], attLV[:], AFT.Exp)
            epB = smp.tile([128, S], F32, tag="sm", name="poe_ep")
            nc.vector.tensor_scalar_add(epB[:], eB[:], EPS)
            prB = smp.tile([128, S], F32, tag="sm", name="poe_pr")
            nc.vector.reciprocal(prB[:], epB[:])
            pmB = smp.tile([128, S], F32, tag="sm", name="poe_pm")
            nc.vector.tensor_tensor(out=pmB[:], in0=prB[:], in1=MsbB[:], op=ALU.mult)
            # joint_var = 1 / (1 + eps + pm0 + pm1) ; joint_mu = jv * (pm.mu sum)
            # (two-input ops need both reads at the same partition base, so
            #  stage the upper halves down to base 0 first)
            pm1z = smp.tile([Z, S], F32, tag="smz", name="pm1z")
            nc.scalar.activation(pm1z[:], pmB[Z:2 * Z, :], AFT.Copy)
            tmp = smp.tile([Z, S], F32, tag="smz")
            nc.vector.scalar_tensor_tensor(
                out=tmp[:], in0=pmB[0:Z, :], scalar=1.0 + EPS, in1=pm1z[:],
                op0=ALU.add, op1=ALU.add,
            )
            jv = smp.tile([Z, S], F32, tag="smz")
            nc.vector.reciprocal(jv[:], tmp[:])
            nB = smp.tile([128, S], F32, tag="sm", name="poe_n")
            nc.vector.tensor_tensor(out=nB[:], in0=pmB[:], in1=attMU[:], op=ALU.mult)
            n1z = smp.tile([Z, S], F32, tag="smz", name="n1z")
            nc.scalar.activation(n1z[:], nB[Z:2 * Z, :], AFT.Copy)
            nsum = smp.tile([Z, S], F32, tag="smz")
            nc.vector.tensor_tensor(out=nsum[:], in0=nB[0:Z, :], in1=n1z[:],
                                    op=ALU.add)
            jmu = smp.tile([Z, S], F16, tag="smz16")
            nc.vector.tensor_tensor(out=jmu[:], in0=jv[:], in1=nsum[:], op=ALU.mult)

            # fused predictors: k=0 joint(jmu), k=1 m0(mu0), k=2 m1(mu1)
            mu1z = smp.tile([Z, S], F16, tag="smz16", name="mu1z")
            nc.scalar.activation(mu1z[:], attMU[Z:2 * Z, :], AFT.Copy)
            zTs = [jmu[:], attMU[0:Z, :], mu1z[:]]
            exB = smp.tile([69, S], F16, tag="smy")
            nc.vector.memset(exB[:], 0.0)
            for k in range(3):
                psa = ps512.tile([128, S], F32, tag="psAgg")
                nc.tensor.matmul(psa[:], spW1[k][:], zTs[k], start=True, stop=True)
                aT = wp.tile([128, S], F16, tag="w512h")
                nc.scalar.activation(
                    aT[:], psa[:], AFT.Prelu, bias=spb1[k][:, 0:1], scale=1.0,
                    alpha=0.25,
                )
                pslg = pssm.tile([Y, S], F32, tag="sm", name=f"pslg{k}")
                nc.tensor.matmul(pslg[:], spW2[k][:], aT[:], start=True, stop=True)
                nc.scalar.activation(exB[32 * k:32 * k + Y, :], pslg[:], AFT.Exp,
                                     bias=spb2[k][:, 0:1], scale=1.0)
            ps3 = pssm.tile([3, S], F32, tag="sm", name="ps3")
            nc.tensor.matmul(ps3[:], selDown[:], exB[:], start=True, stop=True)
            rec3 = smp.tile([3, S], F16, tag="smy3")
            with nc.allow_low_precision(reason="softmax denom fp16 ~5e-4 rel"):
                nc.vector.reciprocal(rec3[:], ps3[:])
            psB = pssm.tile([69, S], F32, tag="sm", name="psB")
            nc.tensor.matmul(psB[:], selUp[:], rec3[:], start=True, stop=True)
            yB = smp.tile([69, S], F32, tag="smy")
            nc.vector.tensor_tensor(out=yB[:], in0=exB[:], in1=psB[:], op=ALU.mult)
            for k in range(3):
                nc.sync.dma_start(out=P["outT"][k * Y:(k + 1) * Y, :],
                                  in_=yB[32 * k:32 * k + Y, :])

        # ---------- emission order (interleave modalities for overlap) ----
        # K_REPS>1 repeats the body for marginal-cost timing (bench only).
        for _ in range(int(os.environ.get("K_REPS", "1"))):
            stage_A(0)
            stage_A(1)
            stage_agg(0, ag1_out[0], b1[0], x1T[0])
            stage_C(0)
            stage_agg(1, ag1_out[1], b1[1], x1T[1])
            stage_C(1)
            stage_agg(0, ag2_out[0], b2[0], x2T[0])
            stage_E(0)
            stage_F(0)
            stage_agg(1, ag2_out[1], b2[1], x2T[1])
            stage_E(1)
            stage_F(1)
            stage_G()


@functools.lru_cache(maxsize=1)
def _get_compiled():
    nc = bacc.Bacc("TRN2", target_bir_lowering=False, debug=False,
                   num_devices=NCORES)
    P = {}
    for m in range(M):
        P[f"adjT{m}"] = nc.dram_tensor(f"adjT{m}", [N, S], F16, kind="ExternalInput").ap()
        P[f"xT{m}"] = nc.dram_tensor(f"xT{m}", [DP, S], F16, kind="ExternalInput").ap()
        P[f"gc1_W{m}"] = nc.dram_tensor(f"gc1_W{m}", [DP, H], F16, kind="ExternalInput").ap()
        P[f"gc1_b{m}"] = nc.dram_tensor(f"gc1_b{m}", [H, 1], F32, kind="ExternalInput").ap()
        P[f"gc2_W{m}"] = nc.dram_tensor(f"gc2_W{m}", [H, H], F16, kind="ExternalInput").ap()
        P[f"gc2_b{m}"] = nc.dram_tensor(f"gc2_b{m}", [H, 1], F32, kind="ExternalInput").ap()
        P[f"gat_W{m}"] = nc.dram_tensor(f"gat_W{m}", [H, F2], F32R, kind="ExternalInput").ap()
        P[f"gat_a{m}"] = nc.dram_tensor(f"gat_a{m}", [2 * F2, 1], F32, kind="ExternalInput").ap()
        P[f"spW1_{m}"] = nc.dram_tensor(f"spW1_{m}", [Z, PH], F16, kind="ExternalInput").ap()
        P[f"spb1_{m}"] = nc.dram_tensor(f"spb1_{m}", [PH, 1], F32, kind="ExternalInput").ap()
        P[f"spW2_{m}"] = nc.dram_tensor(f"spW2_{m}", [PH, Y], F16, kind="ExternalInput").ap()
        P[f"spb2_{m}"] = nc.dram_tensor(f"spb2_{m}", [Y, 1], F32, kind="ExternalInput").ap()
    P["jpW1"] = nc.dram_tensor("jpW1", [Z, PH], F16, kind="ExternalInput").ap()
    P["jpb1"] = nc.dram_tensor("jpb1", [PH, 1], F32, kind="ExternalInput").ap()
    P["jpW2"] = nc.dram_tensor("jpW2", [PH, Y], F16, kind="ExternalInput").ap()
    P["jpb2"] = nc.dram_tensor("jpb2", [Y, 1], F32, kind="ExternalInput").ap()
    P["maskT"] = nc.dram_tensor("maskT", [1, M * S], F32, kind="ExternalInput").ap()
    P["selDown"] = nc.dram_tensor("selDown", [69, 3], F16, kind="ExternalInput").ap()
    P["selUp"] = nc.dram_tensor("selUp", [3, 69], F16, kind="ExternalInput").ap()
    P["outT"] = nc.dram_tensor("outT", [3 * Y, S], F32, kind="ExternalOutput").ap()

    with tile.TileContext(nc) as tc:
        _emit(nc, tc, P)
    nc.compile()
    nc.m = get_hw_module(nc.m)
    return nc


def _shard_inputs(inputs):
    f = np.float32
    h = np.float16
    in_maps = []
    pad_w = []
    for m in range(M):
        w = np.zeros((DP, H), h)
        w[:D, :] = np.asarray(inputs[f"gc1_W{m}"], f)
        pad_w.append(np.ascontiguousarray(w))
    for c in range(NCORES):
        r0, r1 = c * S, (c + 1) * S
        im = {}
        for m in range(M):
            im[f"adjT{m}"] = np.ascontiguousarray(
                np.asarray(inputs[f"adj{m}"], f)[r0:r1, :].T.astype(h))
            xp = np.zeros((DP, S), h)
            xp[:D, :] = np.asarray(inputs[f"x{m}"], f)[r0:r1, :].T
            im[f"xT{m}"] = xp
            im[f"gc1_W{m}"] = pad_w[m]
            im[f"gc1_b{m}"] = np.asarray(inputs[f"gc1_b{m}"], f).reshape(H, 1)
            im[f"gc2_W{m}"] = np.ascontiguousarray(np.asarray(inputs[f"gc2_W{m}"], f).astype(h))
            im[f"gc2_b{m}"] = np.asarray(inputs[f"gc2_b{m}"], f).reshape(H, 1)
            im[f"gat_W{m}"] = np.ascontiguousarray(np.asarray(inputs[f"gat_W{m}"], f))
            im[f"gat_a{m}"] = np.ascontiguousarray(np.asarray(inputs[f"gat_a{m}"], f))
            im[f"spW1_{m}"] = np.ascontiguousarray(np.asarray(inputs[f"spW1_{m}"], f).astype(h))
            im[f"spb1_{m}"] = np.asarray(inputs[f"spb1_{m}"], f).reshape(PH, 1)
            im[f"spW2_{m}"] = np.ascontiguousarray(np.asarray(inputs[f"spW2_{m}"], f).astype(h))
            im[f"spb2_{m}"] = np.asarray(inputs[f"spb2_{m}"], f).reshape(Y, 1)
        im["jpW1"] = np.ascontiguousarray(np.asarray(inputs["jpW1"], f).astype(h))
        im["jpb1"] = np.asarray(inputs["jpb1"], f).reshape(PH, 1)
        im["jpW2"] = np.ascontiguousarray(np.asarray(inputs["jpW2"], f).astype(h))
        im["jpb2"] = np.asarray(inputs["jpb2"], f).reshape(Y, 1)
        im["maskT"] = np.ascontiguousarray(
            np.asarray(inputs["mask"], f)[r0:r1, :].T.reshape(1, M * S))
        sel = np.zeros((69, 3), h)
        for k in range(3):
            sel[32 * k:32 * k + Y, k] = 1.0
        im["selDown"] = sel
        im["selUp"] = np.ascontiguousarray(sel.T)
        in_maps.append(im)
    return in_maps


def run(inputs, trace=False):
    nc = _get_compiled()
    in_maps = _shard_inputs(inputs)
    res = run_bass_kernel_spmd(nc, in_maps, list(range(NCORES)), trace=trace)
    out = np.zeros((N, 3 * Y), np.float32)
    for c in range(NCORES):
        out[c * S:(c + 1) * S, :] = res.results[c]["outT"].T
    return out, res


def kernel(**inputs):
    out, _ = run(inputs)
    return out


# revision 34
# speedup vs baseline: 1.0268x; 1.0268x over previous
"""Trainium2 Bass kernel for nn_GCFNN (2-modality GCN+GAT VAE-ish net).

Strategy: row-shard the node dim N=4096 across 8 cores (512 rows each).
Each core holds adj[rows_c].T ([4096, 512], node-j on partitions) resident in
SBUF in fp16. Per GCN layer: cores compute their support slice x_c @ W (fp16),
AllGather it ([N, 256] fp16), then aggregate transposed:
    x1T[f, i] = sum_j s[j, f] * adjT[j, i]
so the bias add + leaky-relu fuse into one ScalarE Prelu. GAT attention is
computed with logits transposed (eT[j, i]); the per-j a2h projection rides in
the h_aug gather's spare column (no separate collective), masked-exp runs as
DVE add + DVE lrelu(max) + Act exp(l-4) [softmax-shift-invariant, fp16-safe]
+ Pool mask-mult, and the masked-exp output pm is directly the lhsT of att@h.
The joint product-of-experts batches both modalities on partitions; the three
predictor MLPs share one fused softmax via indicator-matrix matmuls. The
[15, 512] per-core result is transposed host-side.
"""

import functools
import os
import sys

import numpy as np

if "/opt/trn_rl_repo" not in sys.path:
    sys.path.insert(0, "/opt/trn_rl_repo")

import concourse.bacc as bacc
import concourse.mybir as mybir
import concourse.tile as tile
from concourse.bass_interp import get_hw_module
from concourse.bass_utils import run_bass_kernel_spmd

N, D, H, F2, Z, Y, PH, M = 4096, 400, 256, 128, 64, 5, 128, 2
NCORES = 8
S = N // NCORES          # 512 rows per core
NJT = N // 128           # 32 j-tiles
NIB = S // 128           # 4 i-blocks
DP = 512                 # D padded to 4*128
NDT = DP // 128          # 4 d-tiles
NFB = H // 128           # 2 feature blocks
HA = 130                 # h_aug cols: 128 h + 1 ones + 1 a2h
ESHIFT = -4.0            # exp(l + ESHIFT): softmax-invariant, keeps fp16 small
EPS = 1e-8

F32 = mybir.dt.float32
F32R = mybir.dt.float32r
F16 = mybir.dt.float16

AFT = mybir.ActivationFunctionType
ALU = mybir.AluOpType


def _emit(nc, tc, P):
    """Emit the whole per-core program. P = dict of DRAM param APs."""
    ag1_in, ag1_out, ag2_in, ag2_out = [], [], [], []
    agh_in, agh_out = [], []
    for m in range(M):
        ag1_in.append(nc.dram_tensor(f"ag1_in{m}", [S, H], F16))
        ag1_out.append(nc.dram_tensor(f"ag1_out{m}", [N, H], F16, addr_space="Shared"))
        ag2_in.append(nc.dram_tensor(f"ag2_in{m}", [S, H], F16))
        ag2_out.append(nc.dram_tensor(f"ag2_out{m}", [N, H], F16, addr_space="Shared"))
        agh_in.append(nc.dram_tensor(f"agh_in{m}", [S, HA], F16))
        agh_out.append(nc.dram_tensor(f"agh_out{m}", [N, HA], F16, addr_space="Shared"))

    rg = [list(range(NCORES))]

    with (
        tc.tile_pool(name="persist", bufs=1) as pp,
        tc.tile_pool(name="stream", bufs=5) as sp,
        tc.tile_pool(name="work", bufs=3) as wp,
        tc.tile_pool(name="att16", bufs=4) as ap16,
        tc.tile_pool(name="pmpool", bufs=3) as pmp,
        tc.tile_pool(name="small", bufs=4) as smp,
        tc.tile_pool(name="ps512", bufs=4, space="PSUM") as ps512,
        tc.tile_pool(name="ps256", bufs=2, space="PSUM") as ps256,
        tc.tile_pool(name="pssm", bufs=2, space="PSUM") as pssm,
    ):
        # ---------- persistent loads (one DMA per tensor) ----------
        adjT, W1, W2, Wg, xTt, biasb, ga16 = [], [], [], [], [], [], []
        for m in range(M):
            t = pp.tile([128, NJT * S], F16, tag=f"adjT{m}", name=f"adjT{m}")
            nc.sync.dma_start(
                out=t[:].rearrange("p (t s) -> p t s", t=NJT),
                in_=P[f"adjT{m}"].rearrange("(t p) s -> p t s", p=128),
            )
            adjT.append(t)

            t = pp.tile([128, NDT * H], F16, tag=f"W1_{m}", name=f"W1_{m}")
            nc.sync.dma_start(
                out=t[:].rearrange("p (t h) -> p t h", t=NDT),
                in_=P[f"gc1_W{m}"].rearrange("(t p) h -> p t h", p=128),
            )
            W1.append(t)

            t = pp.tile([128, NFB * H], F16, tag=f"W2_{m}", name=f"W2_{m}")
            nc.sync.dma_start(
                out=t[:].rearrange("p (t h) -> p t h", t=NFB),
                in_=P[f"gc2_W{m}"].rearrange("(t p) h -> p t h", p=128),
            )
            W2.append(t)

            t = pp.tile([128, NFB * F2], F32R, tag=f"Wg_{m}", name=f"Wg_{m}")
            nc.sync.dma_start(
                out=t[:].rearrange("p (t f) -> p t f", t=NFB),
                in_=P[f"gat_W{m}"].rearrange("(t p) f -> p t f", p=128),
            )
            Wg.append(t)

            t = pp.tile([128, NDT * S], F16, tag=f"xTt{m}", name=f"xTt{m}")
            nc.sync.dma_start(
                out=t[:].rearrange("p (t s) -> p t s", t=NDT),
                in_=P[f"xT{m}"].rearrange("(t p) s -> p t s", p=128),
            )
            xTt.append(t)

            # bias blob: cols 0:2 gc1_b, 2:4 gc2_b, 4:6 gat_a (all f32)
            t = pp.tile([128, 6], F32, tag=f"biasb{m}", name=f"biasb{m}")
            nc.sync.dma_start(out=t[:], in_=P[f"bias{m}"][:, :])
            biasb.append(t)
            t16 = pp.tile([128, 2], F16, tag=f"ga16_{m}", name=f"ga16_{m}")
            nc.vector.tensor_copy(t16[:], t[:, 4:6])
            ga16.append(t16)


        # predictor weights, packed: order j(=joint), 0, 1 -> rows 0:5/5:10/10:15
        spW1B = pp.tile([Z, 3 * PH], F16, tag="spW1B")
        nc.sync.dma_start(out=spW1B[:], in_=P["spW1B"][:, :])
        spW2B = pp.tile([PH, 3 * Y], F16, tag="spW2B")
        nc.sync.dma_start(out=spW2B[:], in_=P["spW2B"][:, :])
        spb1B = pp.tile([PH, 3], F32, tag="spb1B")
        nc.sync.dma_start(out=spb1B[:], in_=P["spb1B"][:, :])
        spb2B = pp.tile([Y, 3], F32, tag="spb2B")
        nc.sync.dma_start(out=spb2B[:], in_=P["spb2B"][:, :])
        spW1 = [spW1B[:, k * PH:(k + 1) * PH] for k in range(3)]
        spW2 = [spW2B[:, k * Y:(k + 1) * Y] for k in range(3)]
        spb1 = [spb1B[:, k:k + 1] for k in range(3)]
        spb2 = [spb2B[:, k:k + 1] for k in range(3)]

        ones_row = pp.tile([1, 128], F16, tag="ones_row")
        nc.vector.memset(ones_row[:], 1.0)
        eshift_b = pp.tile([128, 1], F32, tag="eshift_b")
        nc.vector.memset(eshift_b[:], ESHIFT)
        # predictor-softmax indicator mats (host constants): predictors live
        # at partition bases 0/32/64 of a [69, S] stack
        selDown = pp.tile([69, 3], F16, tag="selDown")
        nc.sync.dma_start(out=selDown[:], in_=P["selDown"][:, :])
        selUp = pp.tile([3, 69], F16, tag="selUp")
        nc.sync.dma_start(out=selUp[:], in_=P["selUp"][:, :])

        # per-modality persistent intermediates
        x1T = [pp.tile([128, NFB * S], F16, tag=f"x1T{m}", name=f"x1T{m}") for m in range(M)]
        x2T = [pp.tile([128, NFB * S], F32R, tag=f"x2T{m}", name=f"x2T{m}") for m in range(M)]
        hT = [pp.tile([128, S], F16, tag=f"hT{m}", name=f"hT{m}") for m in range(M)]
        a1hb = [pp.tile([128, S], F16, tag=f"a1hb{m}", name=f"a1hb{m}") for m in range(M)]
        a2hb = [pp.tile([128, NJT], F32, tag=f"a2hb{m}", name=f"a2hb{m}") for m in range(M)]
        # attention out, batched across modalities on partitions:
        # attMU rows 64m:64m+64 = mu_m ; attLV rows 64m:64m+64 = logvar_m
        attMU = pp.tile([128, S], F16, tag="attMU")
        attLV = pp.tile([128, S], F16, tag="attLV")

        # ---------- stage A: support1 = x @ W1 (sharded), AllGather ----------
        def stage_A(m):
            sbB = wp.tile([128, NIB * H], F16, tag="sbB", bufs=2)
            for ib in range(NIB):
                ps = ps256.tile([128, H], F32, tag="psA")
                for k in range(NDT):
                    nc.tensor.matmul(
                        ps[:],
                        xTt[m][:, k * S + ib * 128: k * S + (ib + 1) * 128],
                        W1[m][:, k * H:(k + 1) * H],
                        start=(k == 0), stop=(k == NDT - 1),
                    )
                nc.vector.tensor_copy(sbB[:, ib * H:(ib + 1) * H], ps[:])
            nc.sync.dma_start(
                out=ag1_in[m].ap().rearrange("(t p) h -> p t h", p=128),
                in_=sbB[:],
            )
            nc.gpsimd.collective_compute(
                "AllGather", ALU.bypass, replica_groups=rg,
                ins=[ag1_in[m].ap().opt()], outs=[ag1_out[m].ap().opt()],
            )

        # ---------- aggregation: outT = prelu(adj @ s + b), transposed -------
        def stage_agg(m, ag_out, bcol, outT):
            CH = 8
            psf = [ps512.tile([128, S], F32, tag="psAgg", name=f"psf{i}")
                   for i in range(NFB)]
            for jc in range(NJT // CH):
                st = sp.tile([128, CH * H], F16, tag="sstream", bufs=2)
                nc.sync.dma_start(
                    out=st[:].rearrange("p (t h) -> p t h", t=CH),
                    in_=ag_out[jc * CH * 128:(jc + 1) * CH * 128, :]
                        .rearrange("(t p) h -> p t h", p=128),
                )
                for jj in range(CH):
                    j = jc * CH + jj
                    for fb in range(NFB):
                        nc.tensor.matmul(
                            psf[fb][:],
                            st[:, jj * H + fb * 128: jj * H + (fb + 1) * 128],
                            adjT[m][:, j * S:(j + 1) * S],
                            start=(j == 0), stop=(j == NJT - 1),
                        )
            for fb in range(NFB):
                nc.scalar.activation(
                    outT[:, fb * S:(fb + 1) * S], psf[fb][:],
                    AFT.Prelu, bias=biasb[m][:, bcol + fb:bcol + fb + 1],
                    scale=1.0, alpha=0.25,
                )

        # ---------- stage C: support2 = x1 @ W2 (sharded), AllGather ---------
        def stage_C(m):
            sbB = wp.tile([128, NIB * H], F16, tag="sbB", bufs=2)
            for ib in range(NIB):
                ps = ps256.tile([128, H], F32, tag="psA")
                for fb in range(NFB):
                    nc.tensor.matmul(
                        ps[:],
                        x1T[m][:, fb * S + ib * 128: fb * S + (ib + 1) * 128],
                        W2[m][:, fb * H:(fb + 1) * H],
                        start=(fb == 0), stop=(fb == NFB - 1),
                    )
                nc.vector.tensor_copy(sbB[:, ib * H:(ib + 1) * H], ps[:])
            nc.sync.dma_start(
                out=ag2_in[m].ap().rearrange("(t p) h -> p t h", p=128),
                in_=sbB[:],
            )
            nc.gpsimd.collective_compute(
                "AllGather", ALU.bypass, replica_groups=rg,
                ins=[ag2_in[m].ap().opt()], outs=[ag2_out[m].ap().opt()],
            )

        # ---------- stage E: h, hT, a-projections, AllGather h_aug ----------
        def stage_E(m):
            # hT[g, i] = sum_f Wg[f, g] x2T[f, i]
            pshT = ps512.tile([128, S], F32, tag="psAgg")
            for fb in range(NFB):
                nc.tensor.matmul(
                    pshT[:],
                    Wg[m][:, fb * F2:(fb + 1) * F2],
                    x2T[m][:, fb * S:(fb + 1) * S],
                    start=(fb == 0), stop=(fb == NFB - 1),
                )
            nc.vector.tensor_copy(hT[m][:], pshT[:])

            # a1h broadcast [128, S]: lhsT = a1 replicated over free dim
            a1rep = smp.tile([128, 128], F16, tag="smh")
            nc.vector.memset(a1rep[:], 0.0)
            nc.vector.tensor_scalar_add(a1rep[:], a1rep[:], biasb[m][:, 4:5])
            psa1 = ps512.tile([128, S], F32, tag="psAgg")
            nc.tensor.matmul(psa1[:], a1rep[:], hT[m][:], start=True, stop=True)
            nc.vector.tensor_copy(a1hb[m][:], psa1[:])

            # a2h per i-block + h blocks -> h_aug bounce (a2h rides col 129)
            psa2 = pssm.tile([128, NIB], F32, tag="sm")
            hcat = wp.tile([128, NIB * HA], F16, tag="hcat", bufs=1)
            hc3 = hcat[:].rearrange("p (t c) -> p t c", t=NIB)
            for ib in range(NIB):
                nc.tensor.matmul(
                    psa2[:, ib:ib + 1],
                    hT[m][:, ib * 128:(ib + 1) * 128],
                    ga16[m][:, 1:2],
                    start=True, stop=True,
                )
                psh = ps256.tile([128, F2], F32, tag="psA")
                for fb in range(NFB):
                    nc.tensor.matmul(
                        psh[:],
                        x2T[m][:, fb * S + ib * 128: fb * S + (ib + 1) * 128],
                        Wg[m][:, fb * F2:(fb + 1) * F2],
                        start=(fb == 0), stop=(fb == NFB - 1),
                    )
                nc.vector.tensor_copy(hcat[:, ib * HA: ib * HA + F2], psh[:])
            nc.vector.memset(hc3[:, :, F2:F2 + 1], 1.0)
            nc.vector.tensor_copy(hc3[:, :, F2 + 1:HA], psa2[:])
            nc.sync.dma_start(
                out=agh_in[m].ap().rearrange("(t p) c -> p t c", p=128),
                in_=hcat[:],
            )
            nc.gpsimd.collective_compute(
                "AllGather", ALU.bypass, replica_groups=rg,
                ins=[agh_in[m].ap().opt()], outs=[agh_out[m].ap().opt()],
            )

        # ---------- stage F: masked attention, transposed ----------
        def stage_F(m):
            CH = 8
            psO = ps512.tile([128, S], F32, tag="psAgg")
            psden = pssm.tile([1, S], F32, tag="sm")
            for jc in range(NJT // CH):
                htc = sp.tile([128, CH * HA], F16, tag="haugst", bufs=2)
                nc.sync.dma_start(
                    out=htc[:].rearrange("p (t c) -> p t c", t=CH),
                    in_=agh_out[m][jc * CH * 128:(jc + 1) * CH * 128, :]
                        .rearrange("(t p) c -> p t c", p=128),
                )
                # a2h bias columns to f32 (DVE scalar operands must be f32)
                nc.vector.tensor_copy(
                    a2hb[m][:, jc * CH:(jc + 1) * CH].rearrange(
                        "p (t o) -> p t o", t=CH),
                    htc[:].rearrange("p (t c) -> p t c", t=CH)[:, :, HA - 1:HA],
                )
                for jj in range(CH):
                    j = jc * CH + jj
                    ht = htc[:, jj * HA:(jj + 1) * HA]
                    # v = a1h_i + a2h_j ; l = lrelu(v) = max(v, .25v) (DVE fp16)
                    v = ap16.tile([128, S], F16, tag="attv")
                    nc.vector.tensor_scalar_add(v[:], a1hb[m][:],
                                                a2hb[m][:, j:j + 1])
                    l = ap16.tile([128, S], F16, tag="attl")
                    nc.vector.scalar_tensor_tensor(
                        out=l[:], in0=v[:], scalar=0.25, in1=v[:],
                        op0=ALU.mult, op1=ALU.max,
                    )
                    # p = exp(l - 4)  (Act; softmax-shift-invariant)
                    p = ap16.tile([128, S], F16, tag="attp")
                    nc.scalar.activation(p[:], l[:], AFT.Exp,
                                         bias=eshift_b[:, 0:1], scale=1.0)
                    # pm = (adj > 0) * p
                    pm = pmp.tile([128, S], F16, tag="att_pm", bufs=3)
                    nc.vector.scalar_tensor_tensor(
                        out=pm[:], in0=adjT[m][:, j * S:(j + 1) * S],
                        scalar=0.0, in1=p[:], op0=ALU.is_gt, op1=ALU.mult,
                    )
                    nc.tensor.matmul(
                        psO[:], ht[:, 0:F2], pm[:],
                        start=(j == 0), stop=(j == NJT - 1),
                    )
                    nc.tensor.matmul(
                        psden[:], ht[:, F2:F2 + 1], pm[:],
                        start=(j == 0), stop=(j == NJT - 1),
                    )
            rec = smp.tile([1, S], F16, tag="smr")
            with nc.allow_low_precision(reason="softmax denom fp16 ~5e-4 rel"):
                nc.vector.reciprocal(rec[:], psden[:])
            psR = ps512.tile([128, S], F32, tag="psAgg")
            nc.tensor.matmul(psR[:], ones_row[:, :], rec[:], start=True, stop=True)
            Rsb = wp.tile([128, S], F32, tag="w512")
            nc.vector.tensor_copy(Rsb[:], psR[:])
            sc = wp.tile([128, S], F32, tag="w512")
            nc.vector.tensor_tensor(out=sc[:], in0=psO[:], in1=Rsb[:], op=ALU.mult)
            nc.scalar.activation(attMU[Z * m:Z * (m + 1), :], sc[0:Z, :],
                                 AFT.Prelu, alpha=0.25)
            nc.scalar.activation(attLV[Z * m:Z * (m + 1), :], sc[Z:2 * Z, :],
                                 AFT.Prelu, alpha=0.25)

        # ---------- stage G: joint PoE (batched) + fused predictors ----------
        def stage_G():
            # masks for both modalities, batched on partitions [128, S]
            MsbB = smp.tile([128, S], F32, tag="sm", name="MsbB")
            for m in range(M):
                nc.sync.dma_start(
                    out=MsbB[Z * m:Z * (m + 1), :],
                    in_=P["maskT"][0:1, m * S:(m + 1) * S].to_broadcast((Z, S)),
                )
            # pm_m = mask_m / (exp(lv_m) + EPS), both modalities at once
            eB = smp.tile([128, S], F32, tag="sm", name="poe_e")
            nc.scalar.activation(eB[:], attLV[:], AFT.Exp)
            epB = smp.tile([128, S], F32, tag="sm", name="poe_ep")
            nc.vector.tensor_scalar_add(epB[:], eB[:], EPS)
            prB = smp.tile([128, S], F32, tag="sm", name="poe_pr")
            nc.vector.reciprocal(prB[:], epB[:])
            pmB = smp.tile([128, S], F32, tag="sm", name="poe_pm")
            nc.vector.tensor_tensor(out=pmB[:], in0=prB[:], in1=MsbB[:], op=ALU.mult)
            # joint_var = 1 / (1 + eps + pm0 + pm1) ; joint_mu = jv * (pm.mu sum)
            # (two-input ops need both reads at the same partition base, so
            #  stage the upper halves down to base 0 first)
            pm1z = smp.tile([Z, S], F32, tag="smz", name="pm1z")
            nc.scalar.activation(pm1z[:], pmB[Z:2 * Z, :], AFT.Copy)
            tmp = smp.tile([Z, S], F32, tag="smz")
            nc.vector.scalar_tensor_tensor(
                out=tmp[:], in0=pmB[0:Z, :], scalar=1.0 + EPS, in1=pm1z[:],
                op0=ALU.add, op1=ALU.add,
            )
            jv = smp.tile([Z, S], F32, tag="smz")
            nc.vector.reciprocal(jv[:], tmp[:])
            nB = smp.tile([128, S], F32, tag="sm", name="poe_n")
            nc.vector.tensor_tensor(out=nB[:], in0=pmB[:], in1=attMU[:], op=ALU.mult)
            n1z = smp.tile([Z, S], F32, tag="smz", name="n1z")
            nc.scalar.activation(n1z[:], nB[Z:2 * Z, :], AFT.Copy)
            nsum = smp.tile([Z, S], F32, tag="smz")
            nc.vector.tensor_tensor(out=nsum[:], in0=nB[0:Z, :], in1=n1z[:],
                                    op=ALU.add)
            jmu = smp.tile([Z, S], F16, tag="smz16")
            nc.vector.tensor_tensor(out=jmu[:], in0=jv[:], in1=nsum[:], op=ALU.mult)

            # fused predictors: k=0 joint(jmu), k=1 m0(mu0), k=2 m1(mu1)
            mu1z = smp.tile([Z, S], F16, tag="smz16", name="mu1z")
            nc.scalar.activation(mu1z[:], attMU[Z:2 * Z, :], AFT.Copy)
            zTs = [jmu[:], attMU[0:Z, :], mu1z[:]]
            exB = smp.tile([69, S], F16, tag="smy")
            nc.vector.memset(exB[:], 0.0)
            for k in range(3):
                psa = ps512.tile([128, S], F32, tag="psAgg")
                nc.tensor.matmul(psa[:], spW1[k][:], zTs[k], start=True, stop=True)
                aT = wp.tile([128, S], F16, tag="w512h")
                nc.scalar.activation(
                    aT[:], psa[:], AFT.Prelu, bias=spb1[k][:, 0:1], scale=1.0,
                    alpha=0.25,
                )
                pslg = pssm.tile([Y, S], F32, tag="sm", name=f"pslg{k}")
                nc.tensor.matmul(pslg[:], spW2[k][:], aT[:], start=True, stop=True)
                nc.scalar.activation(exB[32 * k:32 * k + Y, :], pslg[:], AFT.Exp,
                                     bias=spb2[k][:, 0:1], scale=1.0)
            ps3 = pssm.tile([3, S], F32, tag="sm", name="ps3")
            nc.tensor.matmul(ps3[:], selDown[:], exB[:], start=True, stop=True)
            rec3 = smp.tile([3, S], F16, tag="smy3")
            with nc.allow_low_precision(reason="softmax denom fp16 ~5e-4 rel"):
                nc.vector.reciprocal(rec3[:], ps3[:])
            psB = pssm.tile([69, S], F32, tag="sm", name="psB")
            nc.tensor.matmul(psB[:], selUp[:], rec3[:], start=True, stop=True)
            yB = smp.tile([69, S], F32, tag="smy")
            nc.vector.tensor_tensor(out=yB[:], in0=exB[:], in1=psB[:], op=ALU.mult)
            for k in range(3):
                nc.sync.dma_start(out=P["outT"][k * Y:(k + 1) * Y, :],
                                  in_=yB[32 * k:32 * k + Y, :])

        # ---------- emission order (interleave modalities for overlap) ----
        # K_REPS>1 repeats the body for marginal-cost timing (bench only).
        for _ in range(int(os.environ.get("K_REPS", "1"))):
            stage_A(0)
            stage_A(1)
            stage_agg(0, ag1_out[0], 0, x1T[0])
            stage_C(0)
            stage_agg(1, ag1_out[1], 0, x1T[1])
            stage_C(1)
            stage_agg(0, ag2_out[0], 2, x2T[0])
            stage_E(0)
            stage_F(0)
            stage_agg(1, ag2_out[1], 2, x2T[1])
            stage_E(1)
            stage_F(1)
            stage_G()


@functools.lru_cache(maxsize=1)
def _get_compiled():
    nc = bacc.Bacc("TRN2", target_bir_lowering=False, debug=False,
                   num_devices=NCORES)
    P = {}
    for m in range(M):
        P[f"adjT{m}"] = nc.dram_tensor(f"adjT{m}", [N, S], F16, kind="ExternalInput").ap()
        P[f"xT{m}"] = nc.dram_tensor(f"xT{m}", [DP, S], F16, kind="ExternalInput").ap()
        P[f"gc1_W{m}"] = nc.dram_tensor(f"gc1_W{m}", [DP, H], F16, kind="ExternalInput").ap()
        P[f"gc2_W{m}"] = nc.dram_tensor(f"gc2_W{m}", [H, H], F16, kind="ExternalInput").ap()
        P[f"gat_W{m}"] = nc.dram_tensor(f"gat_W{m}", [H, F2], F32R, kind="ExternalInput").ap()
        P[f"bias{m}"] = nc.dram_tensor(f"bias{m}", [128, 6], F32, kind="ExternalInput").ap()
    P["spW1B"] = nc.dram_tensor("spW1B", [Z, 3 * PH], F16, kind="ExternalInput").ap()
    P["spW2B"] = nc.dram_tensor("spW2B", [PH, 3 * Y], F16, kind="ExternalInput").ap()
    P["spb1B"] = nc.dram_tensor("spb1B", [PH, 3], F32, kind="ExternalInput").ap()
    P["spb2B"] = nc.dram_tensor("spb2B", [Y, 3], F32, kind="ExternalInput").ap()
    P["maskT"] = nc.dram_tensor("maskT", [1, M * S], F32, kind="ExternalInput").ap()
    P["selDown"] = nc.dram_tensor("selDown", [69, 3], F16, kind="ExternalInput").ap()
    P["selUp"] = nc.dram_tensor("selUp", [3, 69], F16, kind="ExternalInput").ap()
    P["outT"] = nc.dram_tensor("outT", [3 * Y, S], F32, kind="ExternalOutput").ap()

    with tile.TileContext(nc) as tc:
        _emit(nc, tc, P)
    nc.compile()
    nc.m = get_hw_module(nc.m)
    return nc


def _shard_inputs(inputs):
    f = np.float32
    h = np.float16
    in_maps = []
    pad_w = []
    for m in range(M):
        w = np.zeros((DP, H), h)
        w[:D, :] = np.asarray(inputs[f"gc1_W{m}"], f)
        pad_w.append(np.ascontiguousarray(w))
    for c in range(NCORES):
        r0, r1 = c * S, (c + 1) * S
        im = {}
        for m in range(M):
            im[f"adjT{m}"] = np.ascontiguousarray(
                np.asarray(inputs[f"adj{m}"], f)[r0:r1, :].T.astype(h))
            xp = np.zeros((DP, S), h)
            xp[:D, :] = np.asarray(inputs[f"x{m}"], f)[r0:r1, :].T
            im[f"xT{m}"] = xp
            im[f"gc1_W{m}"] = pad_w[m]
            im[f"gc2_W{m}"] = np.ascontiguousarray(np.asarray(inputs[f"gc2_W{m}"], f).astype(h))
            im[f"gat_W{m}"] = np.ascontiguousarray(np.asarray(inputs[f"gat_W{m}"], f))
            bb = np.zeros((128, 6), f)
            bb[:, 0:2] = np.asarray(inputs[f"gc1_b{m}"], f).reshape(2, 128).T
            bb[:, 2:4] = np.asarray(inputs[f"gc2_b{m}"], f).reshape(2, 128).T
            bb[:, 4:6] = np.asarray(inputs[f"gat_a{m}"], f).reshape(2, 128).T
            im[f"bias{m}"] = bb
        im["spW1B"] = np.ascontiguousarray(np.hstack([
            np.asarray(inputs["jpW1"], f), np.asarray(inputs["spW1_0"], f),
            np.asarray(inputs["spW1_1"], f)]).astype(h))
        im["spW2B"] = np.ascontiguousarray(np.hstack([
            np.asarray(inputs["jpW2"], f), np.asarray(inputs["spW2_0"], f),
            np.asarray(inputs["spW2_1"], f)]).astype(h))
        im["spb1B"] = np.ascontiguousarray(np.stack([
            np.asarray(inputs["jpb1"], f), np.asarray(inputs["spb1_0"], f),
            np.asarray(inputs["spb1_1"], f)], axis=1))
        im["spb2B"] = np.ascontiguousarray(np.stack([
            np.asarray(inputs["jpb2"], f), np.asarray(inputs["spb2_0"], f),
            np.asarray(inputs["spb2_1"], f)], axis=1))
        im["maskT"] = np.ascontiguousarray(
            np.asarray(inputs["mask"], f)[r0:r1, :].T.reshape(1, M * S))
        sel = np.zeros((69, 3), h)
        for k in range(3):
            sel[32 * k:32 * k + Y, k] = 1.0
        im["selDown"] = sel
        im["selUp"] = np.ascontiguousarray(sel.T)
        in_maps.append(im)
    return in_maps


def run(inputs, trace=False):
    nc = _get_compiled()
    in_maps = _shard_inputs(inputs)
    res = run_bass_kernel_spmd(nc, in_maps, list(range(NCORES)), trace=trace)
    out = np.zeros((N, 3 * Y), np.float32)
    for c in range(NCORES):
        out[c * S:(c + 1) * S, :] = res.results[c]["outT"].T
    return out, res


def kernel(**inputs):
    out, _ = run(inputs)
    return out
